# revision 77
# baseline (speedup 1.0000x reference)
"""Trainium2 Bass kernel for per-hyperedge MLP attention (gnn_message_passing).

reference semantics (E=8192 hyperedges, K=32 nodes/edge, D=128, H=64):
    x_he = X[idx]                                     # [E,K,D] gather
    h    = leaky_relu(einsum('ekd,ehd->ekh', x_he, W1) + b1)
    w    = einsum('ekh,eh->ek', h, W2) + b2
    beta = softmax(w, axis=1)
    z    = einsum('ek,ekd->ed', beta, x_he)
    Z    = tanh(leaky_relu(z))
    return Z, beta

Strategy: shard E across 8 cores (1024 edges each; no collectives needed).
Each core gets a host-deduplicated compact node table (Xc, bf16) plus int16
rank lists; the device gathers its 4096 rows/chunk with dma_gather (duplicates
and the (e,k)->slot permutation are resolved on-device).  All matmuls in bf16
with f32 PSUM accumulation.  Modeled per-core exec (TRN2 cost model): ~70us,
near the ~55-60us aggregate DMA floor (W1 16.8MB + gather 8.4MB per core).
leaky_relu(u) = 0.505*u + 0.495*|u| lets the W2 contraction split into
  w = sum_h W2'*|u|  (ACT Abs pass + sign-masked matmul)   [W2' = 0.495*W2]
    + sum_d v1'*x    (v1' = 0.505 * W2^T @ W1, host precomputed)
    + const(e)       (softmax-invariant -> dropped, incl. b2)
"""

import sys
import numpy as np

sys.path.insert(0, "/opt/trn_rl_repo")

import ml_dtypes

BF16 = ml_dtypes.bfloat16

N_NODES = 100000
D = 128          # hidden_channels
H = 64           # hidden size of edge MLP
E = 8192         # hyperedges
K = 32           # nodes per hyperedge
NCORES = 8
E_LOCAL = E // NCORES          # 1024
CHUNK = 128                    # edges per chunk
NCHUNK = E_LOCAL // CHUNK      # 8
NPAIR = CHUNK // 2             # 64 pairs per chunk
NGROUP = CHUNK // 4            # 32 groups (4 edges) per chunk
NSUB = 4                       # gather / W1T sub-tiles per chunk
GSUB = NGROUP // NSUB          # groups per sub-tile (8)
NC_TABLE = 32768               # per-core compact node-table rows (int16 ranks)

_NC_CACHE = {}


def build_bass(nchunk=NCHUNK):
    """Build the per-core SPMD Bass graph (identical on all 8 cores)."""
    from concourse import bass, bacc, mybir
    from concourse.tile import TileContext

    f32 = mybir.dt.float32
    bf16 = mybir.dt.bfloat16
    i16 = mybir.dt.int16
    AF = mybir.ActivationFunctionType

    e_local = nchunk * CHUNK

    nc = bacc.Bacc("TRN2", target_bir_lowering=False, debug=False)

    # ---- DRAM parameters (per-core shard) ----
    # Xc: per-core compact node table (host-deduped rows of X, bf16).
    # idx16: per-chunk int16 rank lists in dma_gather's 16-partition wrap,
    #        replicated to 128 partitions (one copy per Q7 core).
    Xc_d = nc.declare_dram_parameter("Xc", [NC_TABLE, D], bf16, isOutput=False)
    idx_d = nc.declare_dram_parameter(
        "idx16", [nchunk * 128, CHUNK * K // 16], i16, isOutput=False
    )
    w1t_d = nc.declare_dram_parameter(
        "w1t", [nchunk * 128, NPAIR * 128], bf16, isOutput=False
    )
    b2q_d = nc.declare_dram_parameter(
        "b2q", [nchunk * 2, NGROUP * 128], bf16, isOutput=False
    )
    sgq_d = nc.declare_dram_parameter("sgq", [nchunk * 128, 128], bf16, isOutput=False)
    v1q_d = nc.declare_dram_parameter("v1q", [nchunk * 128, 128], bf16, isOutput=False)
    identb_d = nc.declare_dram_parameter("identb", [128, 128], bf16, isOutput=False)
    identf_d = nc.declare_dram_parameter("identf", [128, 128], f32, isOutput=False)
    ist4_d = nc.declare_dram_parameter("ist4", [32, 128], bf16, isOutput=False)
    qmask_d = nc.declare_dram_parameter("qmask", [128, 128], f32, isOutput=False)
    smask_d = nc.declare_dram_parameter("smask", [128, 128], f32, isOutput=False)
    ind2_d = nc.declare_dram_parameter("ind2", [2, 128], bf16, isOutput=False)
    Z_d = nc.declare_dram_parameter("Zout", [e_local, D], f32, isOutput=True)
    B_d = nc.declare_dram_parameter("Bout", [e_local, K], f32, isOutput=True)

    with TileContext(nc) as tc:
        with (
            tc.tile_pool(name="const", bufs=1) as constp,
            tc.tile_pool(name="idx", bufs=3) as idxp,
            tc.tile_pool(name="xg", bufs=8) as xgp,
            tc.tile_pool(name="w1", bufs=3) as w1p,
            tc.tile_pool(name="meta", bufs=2) as metap,
            tc.tile_pool(name="xt", bufs=10) as xtp,
            tc.tile_pool(name="uw", bufs=6) as uwp,
            tc.tile_pool(name="sm", bufs=2) as smp,
            tc.tile_pool(name="psx", bufs=2, space="PSUM") as psx,
            tc.tile_pool(name="psh", bufs=2, space="PSUM") as psh,
            tc.tile_pool(name="psw", bufs=2, space="PSUM") as psw,
            tc.tile_pool(name="psz", bufs=1, space="PSUM") as psz,
            tc.tile_pool(name="pse", bufs=1, space="PSUM") as pse,
        ):
            identb = constp.tile([128, 128], bf16)
            nc.scalar.dma_start(out=identb[:], in_=identb_d[:])
            identf = constp.tile([128, 128], f32)
            nc.scalar.dma_start(out=identf[:], in_=identf_d[:])
            ist4 = constp.tile([32, 128], bf16)
            nc.scalar.dma_start(out=ist4[:], in_=ist4_d[:])
            qmask = constp.tile([128, 128], f32)
            nc.scalar.dma_start(out=qmask[:], in_=qmask_d[:])
            smask = constp.tile([128, 128], f32)
            nc.scalar.dma_start(out=smask[:], in_=smask_d[:])
            ind2 = constp.tile([2, 128], bf16)
            nc.scalar.dma_start(out=ind2[:], in_=ind2_d[:])

            for c in range(nchunk):
                r0 = c * 128
                idx_t = idxp.tile([128, CHUNK * K // 16], i16, tag="idx")
                nc.gpsimd.dma_start(
                    out=idx_t[:], in_=idx_d[r0 : r0 + 128, :]
                )
                b2q = metap.tile([2, NGROUP * 128], bf16, tag="b2q")
                nc.gpsimd.dma_start(out=b2q[:], in_=b2q_d[2 * c : 2 * c + 2, :])
                sgq = metap.tile([128, 128], bf16, tag="sgq")
                nc.scalar.dma_start(out=sgq[:], in_=sgq_d[r0 : r0 + 128, :])
                v1q = metap.tile([128, 128], bf16, tag="v1q")
                nc.scalar.dma_start(out=v1q[:], in_=v1q_d[r0 : r0 + 128, :])
                # chunk gather: 4 dma_gathers x 1024 rows (HW caps num_idxs
                # between 1024 and 2048 - larger wedges the exec unit)
                xgC = xgp.tile([128, NGROUP * 128], bf16, tag="xg")
                for sub in range(4):
                    nc.gpsimd.dma_gather(
                        out_ap=xgC[:, sub * 1024 : (sub + 1) * 1024].rearrange(
                            "p (g d) -> p g d", g=8
                        ),
                        in_ap=Xc_d[:],
                        idxs_ap=idx_t[:, sub * 64 : (sub + 1) * 64],
                        num_idxs=1024,
                        num_idxs_reg=1024,
                        elem_size=D,
                    )
                xg = [xgC[:, g * 128 : (g + 1) * 128] for g in range(NGROUP)]
                w1t = []      # NSUB tiles [128, GSUB*2*128] bf16
                for s in range(NSUB):
                    w1_s = w1p.tile([128, GSUB * 2 * 128], bf16, tag=f"w1{s}")
                    nc.sync.dma_start(
                        out=w1_s[:],
                        in_=w1t_d[
                            r0 : r0 + 128,
                            s * GSUB * 256 : (s + 1) * GSUB * 256,
                        ],
                    )
                    w1t.append(w1_s)

                # ---- per-group compute: transpose, L1(+bias), |.|, L2 ----
                # 4 groups per iteration share one [128, 512] PSUM h-tile so a
                # single ACT Abs covers them (bias/scale live in the matmuls).
                wT_ps = psw.tile([128, 128], f32, tag="wT")
                for q4 in range(NGROUP // 4):
                    xT_list = []
                    for v in range(2):
                        xT_ps = psx.tile([128, 256], bf16, tag="xT")
                        xT_s = xtp.tile([128, 256], bf16, tag="xT")
                        for half in range(2):
                            nc.tensor.transpose(
                                out=xT_ps[:, half * 128 : half * 128 + 128],
                                in_=xg[4 * q4 + 2 * v + half],
                                identity=identb[:],
                            )
                        nc.vector.tensor_copy(out=xT_s[:], in_=xT_ps[:])
                        xT_list.append(xT_s)
                    hp8 = psh.tile([128, 512], f32, tag="hp")
                    for gi in range(4):
                        g = 4 * q4 + gi
                        s, gs = divmod(g, GSUB)
                        xT_s = xT_list[gi // 2]
                        xoff = (gi % 2) * 128
                        hoff = gi * 128
                        nc.tensor.matmul(
                            out=hp8[:, hoff : hoff + 128],
                            lhsT=b2q[:, g * 128 : (g + 1) * 128],
                            rhs=ind2[:],
                            start=True,
                            stop=False,
                            skip_group_check=True,
                        )
                        for jl in range(2):
                            nc.tensor.matmul(
                                out=hp8[:, hoff + jl * 64 : hoff + (jl + 1) * 64],
                                lhsT=w1t[s][
                                    :, (2 * gs + jl) * 128 : (2 * gs + jl + 1) * 128
                                ],
                                rhs=xT_s[:, xoff + jl * 64 : xoff + (jl + 1) * 64],
                                start=False,
                                stop=True,
                                skip_group_check=True,
                            )
                    # uw = |W2' * (x W1^T + b1)| for 4 groups in one ACT op
                    uw8 = uwp.tile([128, 512], bf16, tag="uw")
                    nc.scalar.activation(out=uw8[:], in_=hp8[:], func=AF.Abs)
                    for gi in range(4):
                        g = 4 * q4 + gi
                        xT_s = xT_list[gi // 2]
                        xoff = (gi % 2) * 128
                        # w^T[:, 4g:4g+4] = v1 (linear) term + sign.|u| term
                        nc.tensor.matmul(
                            out=wT_ps[:, 4 * g : 4 * g + 4],
                            lhsT=xT_s[:, xoff : xoff + 128],
                            rhs=v1q[:, 4 * g : 4 * g + 4],
                            start=True,
                            stop=False,
                        )
                        nc.tensor.matmul(
                            out=wT_ps[:, 4 * g : 4 * g + 4],
                            lhsT=uw8[:, gi * 128 : (gi + 1) * 128],
                            rhs=sgq[:, 4 * g : 4 * g + 4],
                            start=False,
                            stop=True,
                        )

                # ---- softmax over k (padded 128-wide, -1e5 on invalid) ----
                wT_s = smp.tile([128, 128], f32, tag="wTs")
                nc.vector.tensor_copy(out=wT_s[:], in_=wT_ps[:])
                T_ps = pse.tile([128, 128], f32, tag="ep")
                nc.tensor.transpose(out=T_ps[:], in_=wT_s[:], identity=identf[:])
                w_m = smp.tile([128, 128], f32, tag="wm")
                nc.vector.tensor_tensor(
                    out=w_m[:], in0=T_ps[:], in1=smask[:], op=mybir.AluOpType.add
                )
                negmax = smp.tile([128, 1], f32, tag="negmax")
                nc.vector.tensor_reduce(
                    out=negmax[:],
                    in_=w_m[:],
                    axis=mybir.AxisListType.X,
                    op=mybir.AluOpType.max,
                    negate=True,
                )
                exps = smp.tile([128, 128], f32, tag="exps")
                sumexp = smp.tile([128, 1], f32, tag="sumexp")
                nc.scalar.activation(
                    out=exps[:],
                    in_=w_m[:],
                    func=AF.Exp,
                    bias=negmax[:, 0:1],
                    accum_out=sumexp[:, 0:1],
                )
                rec = smp.tile([128, 1], f32, tag="rec")
                nc.vector.reciprocal(out=rec[:], in_=sumexp[:])
                # compact-then-normalize: sum the four 32-wide phases first,
                # then scale the [128, 32] result (beta_q never materialized)
                beta_c0 = smp.tile([128, 32], f32, tag="betac0")
                nc.vector.tensor_reduce(
                    out=beta_c0[:],
                    in_=exps[:].rearrange("p (s k) -> p k s", s=4),
                    axis=mybir.AxisListType.X,
                    op=mybir.AluOpType.add,
                )
                beta_c = smp.tile([128, 32], f32, tag="betac")
                nc.vector.tensor_scalar(
                    out=beta_c[:],
                    in0=beta_c0[:],
                    scalar1=rec[:, 0:1],
                    scalar2=None,
                    op0=mybir.AluOpType.mult,
                )
                nc.sync.dma_start(out=B_d[r0 : r0 + 128, :], in_=beta_c[:])

                # ---- block-diag beta for the weighted reduce ----
                bT_ps = pse.tile([32, 128], f32, tag="ep")
                nc.tensor.transpose(out=bT_ps[:], in_=beta_c[:], identity=identf[:])
                bT_s = smp.tile([32, 128], bf16, tag="bTs")
                nc.vector.tensor_copy(out=bT_s[:], in_=bT_ps[:])
                bT4_ps = pse.tile([128, 128], f32, tag="ep")
                nc.tensor.matmul(
                    out=bT4_ps[:], lhsT=ist4[:], rhs=bT_s[:], start=True, stop=True
                )
                beta_bd = smp.tile([128, 128], bf16, tag="betabd")
                nc.vector.tensor_tensor(
                    out=beta_bd[:], in0=bT4_ps[:], in1=qmask[:],
                    op=mybir.AluOpType.mult,
                )

                # ---- z^T = sum_k beta * x  (per 4-edge group) ----
                zT_ps = psz.tile([128, 128], f32, tag="zT")
                for g in range(NGROUP):
                    nc.tensor.matmul(
                        out=zT_ps[:, 4 * g : 4 * g + 4],
                        lhsT=xg[g],
                        rhs=beta_bd[:, 4 * g : 4 * g + 4],
                        start=True,
                        stop=True,
                    )

                # ---- Z = tanh(leaky(z)) ; transpose back to [e, d] ----
                za = smp.tile([128, 128], f32, tag="za")
                nc.scalar.activation(out=za[:], in_=zT_ps[:], func=AF.Abs, scale=0.495)
                zs = smp.tile([128, 128], f32, tag="zs")
                nc.vector.tensor_scalar(
                    out=zs[:], in0=zT_ps[:], scalar1=0.505, scalar2=None,
                    op0=mybir.AluOpType.mult,
                )
                zl = smp.tile([128, 128], f32, tag="zl")
                nc.vector.tensor_tensor(
                    out=zl[:], in0=za[:], in1=zs[:], op=mybir.AluOpType.add
                )
                zt = smp.tile([128, 128], f32, tag="zt")
                nc.scalar.activation(out=zt[:], in_=zl[:], func=AF.Tanh)
                Z_ps = pse.tile([128, 128], f32, tag="ep")
                nc.tensor.transpose(out=Z_ps[:], in_=zt[:], identity=identf[:])
                Z_s = smp.tile([128, 128], f32, tag="Zs")
                nc.vector.tensor_copy(out=Z_s[:], in_=Z_ps[:])
                nc.sync.dma_start(out=Z_d[r0 : r0 + 128, :], in_=Z_s[:])

    nc.compile()
    return nc


# ---------------------------------------------------------------------------
# host-side input prep
# ---------------------------------------------------------------------------

def _consts():
    identb = np.eye(128, dtype=np.float32).astype(BF16)
    identf = np.eye(128, dtype=np.float32)
    ist4 = np.tile(np.eye(32, dtype=np.float32), (1, 4)).astype(BF16)  # [32,128]
    p = np.arange(128)
    ecol = np.arange(128)
    qmask = (p[:, None] // 32 == ecol[None, :] % 4).astype(np.float32)
    smask = np.where(
        (np.arange(128)[None, :] // 32) == (np.arange(128)[:, None] % 4),
        np.float32(0.0),
        np.float32(-1e5),
    )
    ind2 = (np.arange(128)[None, :] // 64 == np.arange(2)[:, None]).astype(
        np.float32
    ).astype(BF16)
    return {
        "identb": identb, "identf": identf, "ist4": ist4,
        "qmask": qmask, "smask": smask, "ind2": ind2,
    }


def prep_core(idx_l, W1_l, b1_l, W2_l, nchunk, Xb):
    """Host-side relayout of one core's E-shard (e_local = nchunk*128 edges).

    Xb: full node table as bf16 [N_NODES, D].  The core's needed rows are
    deduplicated into a compact table Xc with int16 ranks; the on-device
    dma_gather resolves duplicates and the (e, k) -> slot permutation.
    """
    e_local = nchunk * 128
    assert idx_l.shape == (e_local, K)

    uniq, inv = np.unique(np.asarray(idx_l).ravel(), return_inverse=True)
    assert len(uniq) <= NC_TABLE - 1, len(uniq)
    Xc = np.zeros((NC_TABLE, D), BF16)
    Xc[: len(uniq)] = Xb[uniq]
    # rank list in chunk order: lst[c, g*128 + el*32 + k] for edge c*128+4g+el,
    # split into 4 sub-gathers of 1024, each in the 16-partition wrap
    # replicated to 128 partitions.
    inv16 = inv.astype(np.int16).reshape(e_local, K)
    lst = inv16.reshape(nchunk, 4, 1024)                             # c, sub, i
    wrap = np.ascontiguousarray(
        lst.reshape(nchunk, 4, 64, 16).transpose(0, 1, 3, 2)
    )                                                                # c, sub, 16, s
    idx16 = np.ascontiguousarray(
        np.tile(wrap, (1, 1, 8, 1)).transpose(0, 2, 1, 3)            # c, 128, sub, s
    ).reshape(nchunk * 128, CHUNK * K // 16)

    # W1T pair layout with the 0.495*W2 scale folded in:
    #   w1t[c*128 + d, j*128 + r*64 + h] = 0.495*W2[e,h]*W1[c*128+2j+r, h, d]
    W2p = 0.495 * W2_l
    w1s = W1_l * W2p[:, :, None]
    w1 = w1s.reshape(nchunk, NPAIR, 2, H, D)                         # c, j, r, h, d
    w1t = np.ascontiguousarray(w1.transpose(0, 4, 1, 2, 3)).reshape(
        nchunk * 128, NPAIR * 2 * H
    ).astype(BF16)

    # bias rows for the per-group bias matmul:
    #   b2q[c, q, g*128 + p] = 0.495*W2[e,h]*b1[e,h], e = c*128+4g+2q+p//64, h=p%64
    bi = (W2p * b1_l).reshape(nchunk, NGROUP, 2, 2, H)               # c, g, q, r, h
    b2q = np.ascontiguousarray(
        bi.transpose(0, 2, 1, 3, 4).reshape(nchunk, 2, NGROUP * 128)
    ).reshape(nchunk * 2, NGROUP * 128).astype(BF16)

    # sign tile: sgq[c*128 + p, 4g + e'] = sign(W2[c*128+4g+e', p%64]) if p//64==e'%2
    sg = np.sign(W2_l).astype(np.float32).reshape(nchunk, NGROUP, 4, H)  # c,g,e',h
    sgq = np.zeros((nchunk, 2, H, NGROUP, 4), np.float32)            # c, rhalf, h, g, e'
    for ep in range(4):
        sgq[:, ep % 2, :, :, ep] = sg[:, :, ep, :].transpose(0, 2, 1)
    sgq = sgq.reshape(nchunk, 128, 128).reshape(nchunk * 128, 128).astype(BF16)

    # v1 tile: v1q[c*128 + d, 4g + e'] = 0.505 * sum_h W2[e,h] W1[e,h,d]
    v1 = 0.505 * np.einsum("eh,ehd->ed", W2_l, W1_l)                 # [e_local, D]
    v1q = np.ascontiguousarray(
        v1.reshape(nchunk, NGROUP * 4, D).transpose(0, 2, 1)
    ).reshape(nchunk * 128, 128).astype(BF16)

    return {
        "Xc": Xc, "idx16": idx16, "w1t": w1t, "b2q": b2q,
        "sgq": sgq, "v1q": v1q,
    }


def kernel(X, idx, W1, b1, W2, b2):
    from concourse.bass_utils import run_bass_kernel_spmd

    X = np.asarray(X, dtype=np.float32)
    idx = np.asarray(idx)
    W1 = np.asarray(W1, dtype=np.float32)
    b1 = np.asarray(b1, dtype=np.float32)
    W2 = np.asarray(W2, dtype=np.float32)
    b2 = np.asarray(b2, dtype=np.float32)

    if "nc" not in _NC_CACHE:
        _NC_CACHE["nc"] = build_bass(NCHUNK)
    nc = _NC_CACHE["nc"]

    consts = _consts()
    Xb = X.astype(BF16)
    in_maps = []
    for core in range(NCORES):
        lo, hi = core * E_LOCAL, (core + 1) * E_LOCAL
        m = prep_core(idx[lo:hi], W1[lo:hi], b1[lo:hi], W2[lo:hi], NCHUNK, Xb)
        m.update(**consts)
        in_maps.append(m)

    res = run_bass_kernel_spmd(nc, in_maps, core_ids=list(range(NCORES)))
    Z = np.concatenate([res.results[i]["Zout"] for i in range(NCORES)], axis=0)
    beta = np.concatenate([res.results[i]["Bout"] for i in range(NCORES)], axis=0)
    return Z.astype(np.float32), beta.astype(np.float32)
